# revision 32
# baseline (speedup 1.0000x reference)
"""Trainium2 Bass kernel for nn_LogicLayer (difflogic LogicLayer forward).

Computation (reference):
    w  = softmax(weights, axis=-1)            # [OUT, 16]
    c  = w @ GATE_M                           # [OUT, 4]
    a  = x[:, idx_a]; b = x[:, idx_b]         # [B, OUT] feature gathers
    out = c0 + c1*a + c2*b + c3*(a*b)

Strategy (8 NeuronCores, feature-parallel, mixed u8/bf16 gather traffic):
  - x in [0,1) is uploaded transposed in TWO precisions: xT8 (u8,
    q=rint(x*255)) for the idx_a gathers and xT16 (bf16) for the idx_b
    gathers. Each core computes OUT/8 = 2048 output features over the
    full batch. Why mixed: DVE runs at half rate when any operand is
    u8, so the b operand (used in tensor_tensor mult) stays bf16, while
    the a operand (used only in ACT activation and DVE tensor_scalar,
    which tolerate u8 cheaply) is u8 to cut gather read traffic.
    The 1/255 dequant for q_a is folded into c1 and c3. Harness gate is
    2e-2 rel err; this path measures ~7e-3.
  - Per output feature, dma_gather pulls the a-row (4 KB) and b-row
    (8 KB) from HBM by int16 index — one descriptor per row.
  - Gate coefficients c0..c3 are computed on-device from `weights`
    (exp on ScalarE, strided-AP reductions + small tensor ops on VectorE).
  - out = (c0 + c1'*qa) + b*(c2 + c3'*qa): ScalarE does the first
    affine (Identity activation with per-partition bias/scale) and — on
    a subset of chunks, for engine balance — the second; DVE does the
    rest (tensor_scalar + two tensor_tensor passes, bf16).
  - Output written as outT [2048, B] bf16 (contiguous 8 KB per
    partition); host casts back to f32 and transposes.
"""

import numpy as np

BATCH, IN_DIM, OUT_DIM = 4096, 16384, 16384
N_CORES = 8
F_CORE = OUT_DIM // N_CORES  # 2048 output features per core
P = 128


def _build_nc(in_dim, feat_core, batch):
    """Build + compile the per-core Bass program (SPMD, identical cores)."""
    from contextlib import ExitStack

    import concourse.bacc as bacc
    import concourse.mybir as mybir
    import concourse.tile as tile

    F32 = mybir.dt.float32
    BF16 = mybir.dt.bfloat16
    U8 = mybir.dt.uint8
    I16 = mybir.dt.int16
    TT = feat_core // P  # feature chunks per core (16)
    mult = mybir.AluOpType.mult
    add = mybir.AluOpType.add
    subtract = mybir.AluOpType.subtract
    Ident = mybir.ActivationFunctionType.Identity

    nc = bacc.Bacc(
        "TRN2", target_bir_lowering=False, debug=False, num_swdge_queues=4
    )
    xT8 = nc.dram_tensor("xT8", [in_dim, batch], U8, kind="ExternalInput")
    xT16 = nc.dram_tensor("xT16", [in_dim, batch], BF16, kind="ExternalInput")
    w = nc.dram_tensor("w", [feat_core, 16], F32, kind="ExternalInput")
    # combined gather indices: per chunk, 128 idx_a then 128 idx_b
    idx = nc.dram_tensor("idx", [P, 2 * feat_core // 16], I16, kind="ExternalInput")
    outT = nc.dram_tensor("outT", [feat_core, batch], BF16, kind="ExternalOutput")

    with tile.TileContext(nc) as tc, ExitStack() as ctx:
        const_pool = ctx.enter_context(tc.tile_pool(name="const", bufs=1))
        g_pool = ctx.enter_context(tc.tile_pool(name="g", bufs=4))
        uv_pool = ctx.enter_context(tc.tile_pool(name="uv", bufs=4))

        # chunk-0 indices in their own tiny tile so the first gather only
        # waits on a 32 B/partition DMA, not the full index load
        idx0_sb = const_pool.tile([P, 16], I16, tag="idx0")
        nc.sync.dma_start(idx0_sb[:], idx[:, 0:16])
        idx_sb = const_pool.tile([P, 2 * feat_core // 16], I16, tag="idx")
        nc.sync.dma_start(idx_sb[:, 16:], idx[:, 16:])

        c0 = const_pool.tile([P, TT], F32, tag="c0")
        c1 = const_pool.tile([P, TT], F32, tag="c1")
        c2 = const_pool.tile([P, TT], F32, tag="c2")
        c3 = const_pool.tile([P, TT], F32, tag="c3")

        # ---------- gate coefficients ----------
        # The setup pool stays open for the kernel's lifetime (~5 KB per
        # partition): closing it frees its SBUF for the later pools, and the
        # scope-exit barrier that enables that reuse was serializing the
        # FIRST gather behind the whole coefficient chain (~8 us of lead-in).
        sp = ctx.enter_context(tc.tile_pool(name="setup", bufs=1))
        if True:
            w_sb = sp.tile([P, TT, 16], F32, tag="wsb")
            nc.sync.dma_start(w_sb[:], w[:].rearrange("(t p) g -> p t g", p=P))
            E = sp.tile([P, TT, 16], F32, tag="E")
            nc.scalar.activation(E[:], w_sb[:], mybir.ActivationFunctionType.Exp)

            su = sp.tile([P, TT], F32, tag="su")
            nc.vector.reduce_sum(su[:], E[:], axis=mybir.AxisListType.X)
            r = sp.tile([P, TT], F32, tag="r")
            nc.vector.reciprocal(r[:], su[:])

            c0u = sp.tile([P, TT], F32, tag="c0u")
            nc.vector.reduce_sum(c0u[:], E[:, :, 8:16], axis=mybir.AxisListType.X)

            E4 = E[:].rearrange("p t (g2 g1) -> p t g2 g1", g1=4)
            a1 = sp.tile([P, TT], F32, tag="a1")
            nc.vector.reduce_sum(a1[:], E4[:, :, 0:2, 2:4], axis=mybir.AxisListType.XY)
            b1 = sp.tile([P, TT], F32, tag="b1")
            nc.vector.reduce_sum(b1[:], E4[:, :, 2:4, 0:2], axis=mybir.AxisListType.XY)
            c1u = sp.tile([P, TT], F32, tag="c1u")
            nc.vector.tensor_tensor(c1u[:], a1[:], b1[:], op=subtract)

            a2 = sp.tile([P, TT], F32, tag="a2")
            nc.vector.reduce_sum(a2[:], E[:, :, 4:8], axis=mybir.AxisListType.X)
            b2 = sp.tile([P, TT], F32, tag="b2")
            nc.vector.reduce_sum(b2[:], E[:, :, 8:12], axis=mybir.AxisListType.X)
            c2u = sp.tile([P, TT], F32, tag="c2u")
            nc.vector.tensor_tensor(c2u[:], a2[:], b2[:], op=subtract)

            # c3 = (E1+E8) + (E11+E13) - (E2+E4) - (E7+E14) - 2*(E6-E9)
            def eg(g):
                return E[:, :, g : g + 1]

            p1 = sp.tile([P, TT, 1], F32, tag="p1")
            nc.vector.tensor_tensor(p1[:], eg(1), eg(8), op=add)
            p2 = sp.tile([P, TT, 1], F32, tag="p2")
            nc.vector.tensor_tensor(p2[:], eg(11), eg(13), op=add)
            n1 = sp.tile([P, TT, 1], F32, tag="n1")
            nc.vector.tensor_tensor(n1[:], eg(2), eg(4), op=add)
            n2 = sp.tile([P, TT, 1], F32, tag="n2")
            nc.vector.tensor_tensor(n2[:], eg(7), eg(14), op=add)
            d6 = sp.tile([P, TT, 1], F32, tag="d6")
            nc.vector.tensor_tensor(d6[:], eg(6), eg(9), op=subtract)
            pp = sp.tile([P, TT, 1], F32, tag="pp")
            nc.vector.tensor_tensor(pp[:], p1[:], p2[:], op=add)
            nn_ = sp.tile([P, TT, 1], F32, tag="nn")
            nc.vector.tensor_tensor(nn_[:], n1[:], n2[:], op=add)
            c3a = sp.tile([P, TT, 1], F32, tag="c3a")
            nc.vector.tensor_tensor(c3a[:], pp[:], nn_[:], op=subtract)
            c3u = sp.tile([P, TT, 1], F32, tag="c3u")
            nc.vector.scalar_tensor_tensor(
                c3u[:], d6[:], -2.0, c3a[:], op0=mult, op1=add
            )

            # fold the u8 dequant scale into the softmax normalization:
            # a = q_a/255 (b stays real-valued bf16), so c1 and c3 scale
            # by 1/255.
            r255 = sp.tile([P, TT], F32, tag="r255")
            nc.vector.tensor_scalar_mul(r255[:], r[:], 1.0 / 255.0)
            nc.vector.tensor_tensor(c0[:], c0u[:], r[:], op=mult)
            nc.vector.tensor_tensor(c1[:], c1u[:], r255[:], op=mult)
            nc.vector.tensor_tensor(c2[:], c2u[:], r[:], op=mult)
            nc.vector.tensor_tensor(c3[:], c3u[:, :, 0], r255[:], op=mult)

        # ---------- main gather + FMA loop ----------
        o_pool = ctx.enter_context(tc.tile_pool(name="o", bufs=4))
        V_ON_ACT = {3, 7, 11}  # v-affine on ScalarE for these chunks (balance)
        for ci in range(TT):
            # idx columns: first 8 are the 128 idx_a, next 8 the 128 idx_b
            isrc = idx0_sb if ci == 0 else idx_sb
            a_t = g_pool.tile([P, 1, batch], U8, tag="ga")
            nc.gpsimd.dma_gather(
                a_t[:], xT8[:], isrc[:, ci * 16 : ci * 16 + 8], 128, 128, batch,
                queue_num=ci % 4,
            )
            # b: u8-gathered + ScalarE-converted on even chunks (saves HBM
            # read), bf16-gathered on odd chunks (saves ScalarE passes)
            if ci % 2 == 0 or ci == 1:
                b8_t = g_pool.tile([P, 1, batch], U8, tag="gb8")
                nc.gpsimd.dma_gather(
                    b8_t[:], xT8[:], isrc[:, ci * 16 + 8 : ci * 16 + 16], 128, 128,
                    batch, queue_num=(ci + 2) % 4,
                )
                bc_t = uv_pool.tile([P, batch], BF16, tag="bc")
                b_v = bc_t[:]
                nc.scalar.activation(
                    b_v, b8_t[:, 0, :], Ident, scale=1.0 / 255.0
                )
            else:
                b_t = g_pool.tile([P, 1, batch], BF16, tag="gb16")
                nc.gpsimd.dma_gather(
                    b_t[:], xT16[:], isrc[:, ci * 16 + 8 : ci * 16 + 16], 128, 128,
                    batch, queue_num=(ci + 2) % 4,
                )
                b_v = b_t[:, 0, :]
            a_v = a_t[:, 0, :]
            cs = slice(ci, ci + 1)
            # u = c0 + c1'*qa ; v = c2 + c3'*qa
            u = uv_pool.tile([P, batch], BF16, tag="u")
            nc.scalar.activation(u[:], a_v, Ident, bias=c0[:, cs], scale=c1[:, cs])
            v = uv_pool.tile([P, batch], BF16, tag="v")
            if ci in V_ON_ACT:
                nc.scalar.activation(
                    v[:], a_v, Ident, bias=c2[:, cs], scale=c3[:, cs]
                )
            else:
                nc.vector.tensor_scalar(v[:], a_v, c3[:, cs], c2[:, cs], mult, add)
            # v = v*b, then o = v+u  (DVE, all-bf16)
            nc.vector.tensor_tensor(v[:], v[:], b_v, op=mult)
            o_t = o_pool.tile([P, batch], BF16, tag="o")
            nc.vector.tensor_tensor(o_t[:], v[:], u[:], op=add)
            nc.sync.dma_start(outT[ci * P : (ci + 1) * P, :], o_t[:])

    nc.compile()
    return nc


def _pack_idx(idx_a, idx_b, feat_lo, feat_hi):
    """Host-side int16 gather-index buffer for one core.

    Per 128-feature chunk: 128 idx_a then 128 idx_b. dma_gather consumes
    index i from partition i%16, column i//16 (replicated across the 8
    groups of 16 partitions).
    """
    cols = []
    for f0 in range(feat_lo, feat_hi, P):
        ids = np.concatenate(
            [idx_a[f0 : f0 + P], idx_b[f0 : f0 + P]]
        ).astype(np.int16)
        blk = ids.reshape(16, 16)  # [col, partition-within-16]
        cols.append(np.tile(blk.T, (P // 16, 1)))  # [128, 16]
    return np.ascontiguousarray(np.concatenate(cols, axis=1))


_NC_CACHE = {}


def _get_nc():
    key = (IN_DIM, F_CORE, BATCH)
    if key not in _NC_CACHE:
        _NC_CACHE[key] = _build_nc(IN_DIM, F_CORE, BATCH)
    return _NC_CACHE[key]


TRACE = False  # set by dev harness to capture an NTFF profile
LAST_RESULT = None


def kernel(x, weights, idx_a, idx_b):
    global LAST_RESULT
    import ml_dtypes
    from concourse.bass_utils import run_bass_kernel_spmd

    x = np.asarray(x, dtype=np.float32)
    weights = np.asarray(weights, dtype=np.float32)
    idx_a = np.asarray(idx_a)
    idx_b = np.asarray(idx_b)

    nc = _get_nc()
    xT8 = np.ascontiguousarray(np.rint(x * 255.0).astype(np.uint8).T)
    xT16 = np.ascontiguousarray(x.astype(ml_dtypes.bfloat16).T)
    in_maps = []
    for k in range(N_CORES):
        lo, hi = k * F_CORE, (k + 1) * F_CORE
        in_maps.append(
            {
                "xT8": xT8,
                "xT16": xT16,
                "w": np.ascontiguousarray(weights[lo:hi]),
                "idx": _pack_idx(idx_a, idx_b, lo, hi),
            }
        )

    res = run_bass_kernel_spmd(nc, in_maps, list(range(N_CORES)), trace=TRACE)
    LAST_RESULT = res
    out = np.empty((BATCH, OUT_DIM), dtype=np.float32)
    for k in range(N_CORES):
        out[:, k * F_CORE : (k + 1) * F_CORE] = (
            res.results[k]["outT"].astype(np.float32).T
        )
    return out

